# revision 1
# baseline (speedup 1.0000x reference)
"""Trainium2 Bass kernel for a ResNet Bottleneck block (inference).

Reference computation (NCHW, N=128, Cin=Cout=1024, width=256, H=W=14):
    out = relu(bn1(conv1x1(x, w1)))          # 1024 -> 256
    out = relu(bn2(conv3x3(out, w2, pad=1))) # 256 -> 256
    out = bn3(conv1x1(out, w3))              # 256 -> 1024
    y   = relu(out + x)

Strategy:
- Data-parallel: batch 128 sharded as 16 images per NeuronCore (8 cores),
  conv/BN params replicated. One NEFF, SPMD via run_bass_kernel_spmd.
- BN folded on host into per-channel weight scale + bias.
- All convs are matmuls on the TensorEngine with channels on the partition
  (contraction) dim. The 3x3 conv uses a zero-padded 16x16 per-image SBUF
  layout; each of the 9 taps is a shifted-window matmul accumulating in PSUM.
- Compute in bf16 (moving+stationary operands), fp32 PSUM accumulation,
  fp32 output. Residual is added from the bf16 x tiles on the VectorEngine;
  bias+ReLU on the ScalarEngine during PSUM eviction.
"""

import sys

if "/opt/trn_rl_repo" not in sys.path:
    sys.path.insert(0, "/opt/trn_rl_repo")

import numpy as np
import ml_dtypes

import concourse.bass as bass
import concourse.bacc as bacc
import concourse.tile as tile
from concourse import mybir
from concourse.bass_utils import run_bass_kernel_spmd

EPS = 1e-5
NCORES = 8
NLOC = 16          # images per core
C_IN = 1024
WIDTH = 256
C_OUT = 1024
HW = 196           # 14*14
PADHW = 256        # 16*16 zero-padded image
P = 128
KB1 = C_IN // P    # 8 k-blocks for conv1 / residual channel blocks
KB2 = WIDTH // P   # 2 k-blocks for conv2/conv3 input
MB3 = C_OUT // P   # 8 m-blocks for conv3 output
NPAIRS = NLOC // 2  # 8 image pairs; N=392 per matmul
NF = 2 * HW        # 392

BF16 = mybir.dt.bfloat16
F32 = mybir.dt.float32
Relu = mybir.ActivationFunctionType.Relu

_cached = {}


def _build():
    """Build + compile the SPMD NEFF (one core's program). Cached."""
    if "nc" in _cached:
        return _cached["nc"]

    nc = bacc.Bacc("TRN2", target_bir_lowering=False, debug=False,
                   num_devices=NCORES)

    xt_d = nc.dram_tensor("xt", [2, KB1, P, NLOC * HW // 2], BF16,
                          kind="ExternalInput")
    # weights pre-arranged host-side as exact SBUF images (partition-major),
    # so each loads with ONE DMA at max descriptor size
    w1_d = nc.dram_tensor("w1t", [P, KB1 * WIDTH], BF16, kind="ExternalInput")
    w2_d = nc.dram_tensor("w2t", [P, 9 * KB2 * WIDTH], BF16,
                          kind="ExternalInput")
    w3_d = nc.dram_tensor("w3t", [P, KB2 * C_OUT], BF16, kind="ExternalInput")
    b_d = nc.dram_tensor("biases", [P, 2 * KB2 + MB3], F32,
                         kind="ExternalInput")
    id_d = nc.dram_tensor("ident", [P, P], BF16, kind="ExternalInput")
    y_d = nc.dram_tensor("y", [MB3, P, NLOC * HW], BF16, kind="ExternalOutput")

    with tile.TileContext(nc) as tc:
        _emit(tc, nc, xt_d, w1_d, w2_d, w3_d, b_d, id_d, y_d)

    nc.compile()
    _cached["nc"] = nc
    return nc


def _emit(tc, nc, xt_d, w1_d, w2_d, w3_d, b_d, id_d, y_d):
    """PE-density-oriented emission.

    - Accumulation chains into the SAME PSUM bank serialize at the matmul
      latency (~329ns for N=392); chains interleaved across banks pipeline
      at the issue rate (~169ns). So every phase runs its contraction loop
      OUTER over 8 concurrently-open PSUM groups (8 banks), group index
      innermost so consecutive matmuls target different banks.
    - DMA *issue* is ~0.6us per dma_start on the issuing engine: inputs are
      consolidated into 13 DMAs on Sync; outputs are staged into [P, 3136]
      tiles and written with 2 DMAs per m-block issued from GpSimd.
    - The residual add runs on the PE as an identity-weight matmul appended
      to each conv3 accumulation group, so eviction is a single
      relu(psum+bias) op, alternating VectorE/ScalarE.
    """
    import contextlib

    Alu = mybir.AluOpType

    def evict_relu_bias(dst, src, bias_ap, on_vector):
        # dst = relu(src + bias)
        if on_vector:
            nc.vector.tensor_scalar(dst, src, bias_ap, 0.0, Alu.add, Alu.max)
        else:
            nc.scalar.activation(dst, src, Relu, bias=bias_ap)

    with contextlib.ExitStack() as ctx:
        const = ctx.enter_context(tc.tile_pool(name="const", bufs=1))
        xpool = ctx.enter_context(tc.tile_pool(name="xpool", bufs=1))
        opool = ctx.enter_context(tc.tile_pool(name="opool", bufs=1))
        psp = ctx.enter_context(tc.tile_pool(name="psp", bufs=8, space="PSUM"))
        evp = ctx.enter_context(tc.tile_pool(name="evp", bufs=2))

        # ---- Loads, in consumption order, one DMA each -------------------
        # DMA engines stripe packets fairly across ALL active transfers, so
        # concurrent DMAs all complete near the end of the aggregate window.
        # Chain the x loads (depth 3) so early tiles finish early and conv1
        # can consume them as they land.
        from concourse.tile import add_dep_helper

        HNF = 4 * NF
        xsb = xpool.tile([P, KB1 * NLOC * HW], BF16, name="xsb", tag="xsb")
        x_tiles = [xsb[:, k * NLOC * HW:(k + 1) * NLOC * HW]
                   for k in range(KB1)]
        # x dram is half-major; load each half as 4 two-k-tile DMAs (784KB):
        # larger transfers saturate DMA bandwidth with fewer chain links.
        xv = xsb[:].rearrange("p (k h c) -> p k h c", k=KB1, h=2)
        x_dmas = []
        for half in range(2):
            eng = nc.sync if half == 0 else nc.gpsimd
            for j in range(KB1 // 2):
                dst = xv[:, 2 * j:2 * j + 2, half, :]
                s = (xt_d.ap()[half][2 * j:2 * j + 2]
                     .rearrange("k p c -> p k c"))
                i = eng.dma_start(dst, s)
                n = len(x_dmas)
                if n >= 2:
                    add_dep_helper(i.ins, x_dmas[n - 2],
                                   reason="x load pacing")
                x_dmas.append(i.ins)

        w1sb = const.tile([P, KB1 * WIDTH], BF16, name="w1sb", tag="w1sb")
        nc.scalar.dma_start(w1sb[:], w1_d.ap())
        w1_t = [w1sb[:, k * WIDTH:(k + 1) * WIDTH] for k in range(KB1)]

        ball = const.tile([P, 2 * KB2 + MB3], F32, name="ball", tag="ball")
        i = nc.scalar.dma_start(ball[:], b_d.ap())
        add_dep_helper(i.ins, x_dmas[0], reason="bias after early x")
        b1_t = ball[:, 0:KB2]
        b2_t = ball[:, KB2:2 * KB2]
        b3_t = ball[:, 2 * KB2:]

        w2sb = const.tile([P, 9 * KB2 * WIDTH], BF16, name="w2sb", tag="w2sb")
        i = nc.gpsimd.dma_start(w2sb[:], w2_d.ap())
        add_dep_helper(i.ins, x_dmas[5], reason="w2 near end of x")
        w2_t = [[w2sb[:, (tap * KB2 + k) * WIDTH:(tap * KB2 + k + 1) * WIDTH]
                 for k in range(KB2)] for tap in range(9)]

        w3sb = const.tile([P, KB2 * C_OUT], BF16, name="w3sb", tag="w3sb")
        i = nc.gpsimd.dma_start(w3sb[:], w3_d.ap())
        add_dep_helper(i.ins, x_dmas[7], reason="w3 after x")
        w3_t = [w3sb[:, k * C_OUT:(k + 1) * C_OUT] for k in range(KB2)]

        id_t = const.tile([P, P], BF16, name="id_t", tag="id_t")
        i = nc.gpsimd.dma_start(id_t[:], id_d.ap())
        add_dep_helper(i.ins, x_dmas[7], reason="ident after x")

        # PE warm-up: the HAM clock gate needs ~3.4us of sustained PE
        # activity to lift the PE from 1.2 to 2.4 GHz. Run dummy matmuls on
        # a scratch tile while the first x DMAs are still in flight.
        scratch = const.tile([P, 512], BF16, name="scratch", tag="scratch")
        nc.gpsimd.memset(scratch[:], 0.0)
        warm_ps = psp.tile([P, 512], F32, name="warm_ps", tag="ps")
        for _ in range(8):
            nc.tensor.matmul(warm_ps[:], scratch[:, 0:P], scratch[:],
                             start=True, stop=True)

        # Zero-padded conv1 output: per image a 16x16 field, payload at
        # rows/cols 1..14. Layout [P, NLOC*256].
        out1 = []
        for m in range(KB2):
            t = opool.tile([P, NLOC * PADHW], BF16, name=f"out1_{m}",
                           tag=f"out1_{m}")
            nc.vector.memset(t[:], 0.0)
            out1.append(t)

        out2 = []
        for m in range(KB2):
            t = opool.tile([P, NLOC * HW], BF16, name=f"out2_{m}",
                           tag=f"out2_{m}")
            out2.append(t)

        def pad_view(k, np_):
            return (out1[k][:, np_ * 2 * PADHW:(np_ + 1) * 2 * PADHW]
                    .rearrange("p (i r c) -> p i r c", i=2, r=16, c=16))

        # ---- conv1 (1x1, 1024->256) + bias + relu -> padded out1 --------
        # Per np-half: 8 open groups (4 npairs x 2 m), contraction k outer.
        for half in range(2):
            nps = [half * 4 + j for j in range(4)]
            grp = {}
            for np_ in nps:
                for m in range(KB2):
                    ps = psp.tile([P, NF], F32, name=f"ps1_{np_}_{m}",
                                  tag="ps")
                    grp[(np_, m)] = ps
            for k in range(KB1):
                for m in range(KB2):
                    for np_ in nps:
                        nc.tensor.matmul(
                            grp[(np_, m)][:],
                            w1_t[k][:, m * P:(m + 1) * P],
                            x_tiles[k][:, np_ * NF:(np_ + 1) * NF],
                            start=(k == 0), stop=(k == KB1 - 1),
                        )
            for np_ in nps:
                for m in range(KB2):
                    dst = pad_view(m, np_)[:, :, 1:15, 1:15]
                    src = (grp[(np_, m)][:]
                           .rearrange("p (i r c) -> p i r c", i=2, r=14, c=14))
                    evict_relu_bias(dst, src, b1_t[:, m:m + 1],
                                    on_vector=(np_ % 2 == 1))

        # ---- conv2 (3x3, 256->256, pad 1) + bias + relu -> out2 ----------
        # Per np-half: 8 open groups, contraction (k, dy, dx) outer.
        for half in range(2):
            nps = [half * 4 + j for j in range(4)]
            grp = {}
            for np_ in nps:
                for m in range(KB2):
                    grp[(np_, m)] = psp.tile([P, NF], F32,
                                             name=f"ps2_{np_}_{m}", tag="ps")
            for idx, (k, dy, dx) in enumerate(
                    (k, dy, dx) for k in range(KB2)
                    for dy in range(3) for dx in range(3)):
                for m in range(KB2):
                    for np_ in nps:
                        rhs = pad_view(k, np_)[:, :, dy:dy + 14, dx:dx + 14]
                        nc.tensor.matmul(
                            grp[(np_, m)][:]
                            .rearrange("p (i r c) -> p i r c", i=2, r=14, c=14),
                            w2_t[dy * 3 + dx][k][:, m * P:(m + 1) * P],
                            rhs,
                            start=(idx == 0), stop=(idx == 17),
                        )
            for np_ in nps:
                for m in range(KB2):
                    evict_relu_bias(out2[m][:, np_ * NF:(np_ + 1) * NF],
                                    grp[(np_, m)][:], b2_t[:, m:m + 1],
                                    on_vector=(np_ % 2 == 1))

        # ---- conv3 (1x1, 256->1024) + bias + residual + relu -> y --------
        # Per m: 8 open groups (npairs), contraction k outer. The residual
        # lands in PSUM via an identity-weight matmul closing most groups;
        # two groups per pass take the DVE/ACT path instead to shave PE
        # work (DVE stt computes (psum+bias)+x, ACT applies relu). The last
        # pass stays all-PE so its eviction tail is a single op per group.
        for m in range(MB3):
            bgrps = {5, 6, 7} if m < MB3 - 1 else set()
            grp = {}
            for np_ in range(NPAIRS):
                grp[np_] = psp.tile([P, NF], F32, name=f"ps3_{np_}", tag="ps")
            for k in range(KB2):
                for np_ in range(NPAIRS):
                    nc.tensor.matmul(
                        grp[np_][:],
                        w3_t[k][:, m * P:(m + 1) * P],
                        out2[k][:, np_ * NF:(np_ + 1) * NF],
                        start=(k == 0), stop=(k == KB2 - 1 and np_ in bgrps),
                    )
            for np_ in range(NPAIRS):
                if np_ not in bgrps:
                    nc.tensor.matmul(
                        grp[np_][:], id_t[:],
                        x_tiles[m][:, np_ * NF:(np_ + 1) * NF],
                        start=False, stop=True,
                    )
            ystage = evp.tile([P, NLOC * HW], BF16, name="ystage",
                              tag="ystage", bufs=3)
            for np_ in range(NPAIRS):
                dst = ystage[:, np_ * NF:(np_ + 1) * NF]
                if np_ in bgrps:
                    tsum = evp.tile([P, NF], F32, name="tsum", tag="tsum",
                                    bufs=4)
                    nc.vector.scalar_tensor_tensor(
                        tsum[:], grp[np_][:], b3_t[:, m:m + 1],
                        x_tiles[m][:, np_ * NF:(np_ + 1) * NF],
                        Alu.add, Alu.add)
                    nc.scalar.activation(dst, tsum[:], Relu, bias=0.0)
                else:
                    evict_relu_bias(dst, grp[np_][:], b3_t[:, m:m + 1],
                                    on_vector=(np_ % 2 == 1))
            nchunk = 4 if m == MB3 - 1 else 2
            CNF = NLOC * HW // nchunk
            for c in range(nchunk):
                nc.sync.dma_start(y_d.ap()[m][:, c * CNF:(c + 1) * CNF],
                                  ystage[:, c * CNF:(c + 1) * CNF])


def _prep(x, w1, g1, b1, m1, v1, w2, g2, b2, m2, v2, w3, g3, b3, m3, v3):
    """Host-side: fold BN, transpose weights to lhsT layouts, shard x."""
    def fold(w, g, b, m, v):
        scale = (g.astype(np.float64) / np.sqrt(v.astype(np.float64) + EPS))
        bias = b.astype(np.float64) - m.astype(np.float64) * scale
        wf = w.astype(np.float64) * scale.reshape(-1, *([1] * (w.ndim - 1)))
        return wf.astype(np.float32), bias.astype(np.float32)

    w1f, bias1 = fold(w1, g1, b1, m1, v1)   # [256,1024,1,1]
    w2f, bias2 = fold(w2, g2, b2, m2, v2)   # [256,256,3,3]
    w3f, bias3 = fold(w3, g3, b3, m3, v3)   # [1024,256,1,1]

    bf = ml_dtypes.bfloat16
    # lhsT SBUF images [P(=ci within kblock), ...]:
    # w1: [k, p, co] -> [p, (k co)]
    w1t = np.ascontiguousarray(
        w1f[:, :, 0, 0].T.reshape(KB1, P, WIDTH).transpose(1, 0, 2)
        .reshape(P, KB1 * WIDTH)).astype(bf)
    # w2: [tap, k, p, co] -> [p, (tap k co)], tap = dy*3+dx
    w2t = np.ascontiguousarray(
        w2f.transpose(2, 3, 1, 0).reshape(9 * KB2, P, WIDTH)
        .transpose(1, 0, 2).reshape(P, 9 * KB2 * WIDTH)).astype(bf)
    # w3: [k, p, co] -> [p, (k co)]
    w3t = np.ascontiguousarray(
        w3f[:, :, 0, 0].T.reshape(KB2, P, C_OUT).transpose(1, 0, 2)
        .reshape(P, KB2 * C_OUT)).astype(bf)

    b1h = bias1.reshape(KB2, P).T                          # [P, 2]
    b2h = bias2.reshape(KB2, P).T                          # [P, 2]
    b3h = bias3.reshape(MB3, P).T                          # [P, 8]
    ball = np.ascontiguousarray(
        np.concatenate([b1h, b2h, b3h], axis=1), dtype=np.float32)

    # x: [128, 1024, 14, 14] -> per core [2(half), KB1, P, NLOC*HW/2] bf16
    xs = (x.reshape(NCORES, NLOC, KB1, P, HW)
          .transpose(0, 2, 3, 1, 4)
          .reshape(NCORES, KB1, P, NLOC * HW)).astype(bf)
    H = NLOC * HW // 2
    xs = np.stack((xs[..., :H], xs[..., H:]), axis=1)  # [cores,2,KB1,P,H]

    common = {"w1t": w1t, "w2t": w2t, "w3t": w3t,
              "biases": ball, "ident": np.eye(P, dtype=np.float32).astype(bf)}
    in_maps = [dict(common, xt=np.ascontiguousarray(xs[i]))
               for i in range(NCORES)]
    return in_maps


def kernel(**inputs):
    inputs = {k: np.asarray(v) for k, v in inputs.items()}
    in_maps = _prep(**inputs)
    nc = _build()
    res = run_bass_kernel_spmd(nc, in_maps, core_ids=list(range(NCORES)))

    y = np.empty((NCORES * NLOC, C_OUT, 14, 14), dtype=np.float32)
    for i in range(NCORES):
        r = np.asarray(res.results[i]["y"], dtype=np.float32)  # [MB3,P,N*HW]
        r = (r.reshape(MB3, P, NLOC, HW)
             .transpose(2, 0, 1, 3)
             .reshape(NLOC, C_OUT, 14, 14))
        y[i * NLOC:(i + 1) * NLOC] = r
    return y



# revision 2
# speedup vs baseline: 1.4182x; 1.4182x over previous
"""Trainium2 Bass kernel for a ResNet Bottleneck block (inference).

Reference computation (NCHW, N=128, Cin=Cout=1024, width=256, H=W=14):
    out = relu(bn1(conv1x1(x, w1)))          # 1024 -> 256
    out = relu(bn2(conv3x3(out, w2, pad=1))) # 256 -> 256
    out = bn3(conv1x1(out, w3))              # 256 -> 1024
    y   = relu(out + x)

Strategy:
- Data-parallel: batch 128 sharded as 16 images per NeuronCore (8 cores),
  conv/BN params replicated. One NEFF, SPMD via run_bass_kernel_spmd.
- BN folded on host into per-channel weight scale + bias.
- All convs run in fp8-e4m3 with MatmulPerfMode.DoubleRow: each matmul
  contracts K=256 (two 128-channel blocks stacked in AP dim 1) at double
  the bf16 MAC rate. Weights/activations are scaled host-side
  (s1=16, s2=64, s3=256) to sit in e4m3's healthy range; the scale is
  unwound for free: relu(s*a) = s*relu(a), so each conv's input scale is
  folded into the next conv's weights, and the final 1/s3 rides the
  eviction op.
- fp32 PSUM accumulation throughout. conv3's residual is added in PSUM
  by a bf16 identity-weight matmul (weights = s3*I) on the bf16 x tiles,
  so conv3 eviction is a single relu-and-scale op per group.
- Per-image-pair layouts use row-interleaved fields (j = 2*row + img) so
  the 3x3 conv's DoubleRow moving operand is a 4-dim AP
  [p, kpair, 28 interleaved rows, 14 cols] over a zero-padded 32x16
  field, same AP rank as a plain per-image window.
- conv3 bias is folded into the bf16 residual tiles host-side (x + b3),
  keeping the conv3 eviction a 2-ALU op on either DVE or ACT.
"""

import sys

if "/opt/trn_rl_repo" not in sys.path:
    sys.path.insert(0, "/opt/trn_rl_repo")

import numpy as np
import ml_dtypes

import concourse.bass as bass
import concourse.bacc as bacc
import concourse.tile as tile
from concourse import mybir
from concourse.bass_utils import run_bass_kernel_spmd

EPS = 1e-5
NCORES = 8
NLOC = 16          # images per core
C_IN = 1024
WIDTH = 256
C_OUT = 1024
HW = 196           # 14*14
P = 128
KB1 = C_IN // P    # 8 input channel blocks
KP1 = KB1 // 2     # 4 DoubleRow channel-block pairs for conv1
KB2 = WIDTH // P   # 2 channel blocks for conv2/conv3 input
MB3 = C_OUT // P   # 8 output channel blocks for conv3
NPAIRS = NLOC // 2  # 8 image pairs; N=392 per matmul
NF = 2 * HW        # 392
FLD = 512          # padded interleaved pair-field: 32 rows x 16 cols

S1, S2, S3 = 16.0, 64.0, 256.0

F8 = mybir.dt.float8e4
BF16 = mybir.dt.bfloat16
F32 = mybir.dt.float32
Relu = mybir.ActivationFunctionType.Relu
DR = mybir.MatmulPerfMode.DoubleRow

_cached = {}


def _build():
    """Build + compile the SPMD NEFF (one core's program). Cached."""
    if "nc" in _cached:
        return _cached["nc"]

    nc = bacc.Bacc("TRN2", target_bir_lowering=False, debug=False,
                   num_devices=NCORES)

    xq_d = nc.dram_tensor("xq", [KB1, P, NLOC * HW], F8, kind="ExternalInput")
    xr_d = nc.dram_tensor("xr", [KB1, P, NLOC * HW], BF16,
                          kind="ExternalInput")
    # weights pre-arranged host-side as exact SBUF images (partition-major),
    # fp8, DoubleRow pair-ordered; each loads with ONE DMA
    w1_d = nc.dram_tensor("w1t", [P, KP1 * 2 * WIDTH], F8,
                          kind="ExternalInput")
    w2_d = nc.dram_tensor("w2t", [P, 9 * KB2 * WIDTH], F8,
                          kind="ExternalInput")
    w3_d = nc.dram_tensor("w3t", [P, KB2 * C_OUT], F8, kind="ExternalInput")
    b_d = nc.dram_tensor("biases", [P, 2 * KB2], F32, kind="ExternalInput")
    id_d = nc.dram_tensor("ident", [P, P], BF16, kind="ExternalInput")
    y_d = nc.dram_tensor("y", [MB3, P, NLOC * HW], BF16, kind="ExternalOutput")

    with tile.TileContext(nc) as tc:
        _emit(tc, nc, xq_d, xr_d, w1_d, w2_d, w3_d, b_d, id_d, y_d)

    nc.compile()
    _cached["nc"] = nc
    return nc


def _emit(tc, nc, xq_d, xr_d, w1_d, w2_d, w3_d, b_d, id_d, y_d):
    """PE-density-oriented emission.

    - Every phase runs its contraction loop OUTER over 8 concurrently-open
      PSUM groups (8 banks), group index innermost, so consecutive matmuls
      target different banks and pipeline at the issue rate.
    - DMA issue is expensive on the issuing engine: inputs are consolidated
      into 13 DMAs across Sync/ACT/GpSimd; outputs are staged and written
      with 2 DMAs per m-block from Sync.
    - Evictions alternate DVE/ACT, one op per group.
    """
    import contextlib

    Alu = mybir.AluOpType

    with contextlib.ExitStack() as ctx:
        const = ctx.enter_context(tc.tile_pool(name="const", bufs=1))
        xpool = ctx.enter_context(tc.tile_pool(name="xpool", bufs=1))
        opool = ctx.enter_context(tc.tile_pool(name="opool", bufs=1))
        psp = ctx.enter_context(tc.tile_pool(name="psp", bufs=8, space="PSUM"))
        evp = ctx.enter_context(tc.tile_pool(name="evp", bufs=2))

        # ---- Loads, in consumption order, few big DMAs ------------------
        from concourse.tile import add_dep_helper

        xq = xpool.tile([P, KB1 * NLOC * HW], F8, name="xq", tag="xq")
        xqv = xq[:].rearrange("p (k n) -> p k n", k=KB1)
        xq_dmas = []
        for j in range(KB1 // 2):
            dst = xqv[:, 2 * j:2 * j + 2, :]
            s = xq_d.ap()[2 * j:2 * j + 2].rearrange("k p c -> p k c")
            i = nc.sync.dma_start(dst, s)
            n = len(xq_dmas)
            if n >= 2:
                add_dep_helper(i.ins, xq_dmas[n - 2], reason="xq load pacing")
            xq_dmas.append(i.ins)

        w1sb = const.tile([P, KP1 * 2 * WIDTH], F8, name="w1sb", tag="w1sb")
        nc.scalar.dma_start(w1sb[:], w1_d.ap())
        w1v = w1sb[:].rearrange("p (j i c) -> p j i c", j=KP1, i=2)

        ball = const.tile([P, 2 * KB2], F32, name="ball", tag="ball")
        i = nc.scalar.dma_start(ball[:], b_d.ap())
        add_dep_helper(i.ins, xq_dmas[0], reason="bias after early xq")
        b1_t = ball[:, 0:KB2]
        b2_t = ball[:, KB2:2 * KB2]

        w2sb = const.tile([P, 9 * KB2 * WIDTH], F8, name="w2sb", tag="w2sb")
        i = nc.gpsimd.dma_start(w2sb[:], w2_d.ap())
        add_dep_helper(i.ins, xq_dmas[1], reason="w2 behind xq")
        w2v = w2sb[:].rearrange("p (t i c) -> p t i c", t=9, i=2)

        # residual x (bf16, with conv3 bias folded in), 4 chunked DMAs
        xr = xpool.tile([P, KB1 * NLOC * HW], BF16, name="xr", tag="xr")
        xrv = xr[:].rearrange("p (k n) -> p k n", k=KB1)
        xr_dmas = []
        for j in range(KB1 // 2):
            dst = xrv[:, 2 * j:2 * j + 2, :]
            s = xr_d.ap()[2 * j:2 * j + 2].rearrange("k p c -> p k c")
            i = nc.gpsimd.dma_start(dst, s)
            n = len(xr_dmas)
            if n == 0:
                add_dep_helper(i.ins, xq_dmas[2], reason="xr after xq bulk")
            if n >= 2:
                add_dep_helper(i.ins, xr_dmas[n - 2], reason="xr load pacing")
            xr_dmas.append(i.ins)

        w3sb = const.tile([P, KB2 * C_OUT], F8, name="w3sb", tag="w3sb")
        i = nc.gpsimd.dma_start(w3sb[:], w3_d.ap())
        add_dep_helper(i.ins, xr_dmas[1], reason="w3 behind xr")
        w3v = w3sb[:].rearrange("p (i c) -> p i c", i=2)

        id_t = const.tile([P, P], BF16, name="id_t", tag="id_t")
        i = nc.gpsimd.dma_start(id_t[:], id_d.ap())
        add_dep_helper(i.ins, xr_dmas[1], reason="ident behind xr")

        # PE warm-up: the clock gate needs ~3.4us of sustained PE activity
        # to lift the PE from 1.2 to 2.4 GHz. Run dummy matmuls on a
        # scratch tile while the first x DMAs are still in flight.
        scratch = const.tile([P, 512], BF16, name="scratch", tag="scratch")
        nc.gpsimd.memset(scratch[:], 0.0)
        warm_ps = psp.tile([P, 512], F32, name="warm_ps", tag="ps")
        for _ in range(8):
            nc.tensor.matmul(warm_ps[:], scratch[:, 0:P], scratch[:],
                             start=True, stop=True)

        # conv1 output: zero-padded row-interleaved pair fields, fp8.
        # Per image pair a 32x16 field (j = 2*padrow + img), payload at
        # j in 2..29, cols 1..14. Layout [P, KB2 * NPAIRS * FLD].
        out1 = opool.tile([P, KB2 * NPAIRS * FLD], F8, name="out1",
                          tag="out1")
        nc.vector.memset(out1[:], 0.0)
        out1v = out1[:].rearrange("p (k q j c) -> p k q j c",
                                  k=KB2, q=NPAIRS, j=32, c=16)

        out2 = opool.tile([P, KB2 * NLOC * HW], F8, name="out2", tag="out2")
        out2v = out2[:].rearrange("p (k n) -> p k n", k=KB2)

        # ---- conv1 (1x1, 1024->256) + bias + relu -> padded out1 --------
        # Per np-half: 8 open groups (4 npairs x 2 m), contraction kp outer.
        # psum columns are (j, c)-ordered; eviction writes the interior of
        # the padded field directly.
        for half in range(2):
            nps = [half * 4 + j for j in range(4)]
            grp = {}
            for np_ in nps:
                for m in range(KB2):
                    grp[(np_, m)] = psp.tile([P, NF], F32,
                                             name=f"ps1_{np_}_{m}", tag="ps")
            for kp in range(KP1):
                for m in range(KB2):
                    for np_ in nps:
                        nc.tensor.matmul(
                            grp[(np_, m)][:],
                            w1v[:, kp, :, m * P:(m + 1) * P],
                            xqv[:, 2 * kp:2 * kp + 2,
                                np_ * NF:(np_ + 1) * NF],
                            start=(kp == 0), stop=(kp == KP1 - 1),
                            perf_mode=DR,
                        )
            for np_ in nps:
                for m in range(KB2):
                    dst = out1v[:, m, np_, 2:30, 1:15]
                    src = (grp[(np_, m)][:]
                           .rearrange("p (j c) -> p j c", j=28))
                    if np_ % 2 == 1:
                        nc.vector.tensor_scalar(dst, src, b1_t[:, m:m + 1],
                                                0.0, Alu.add, Alu.max)
                    else:
                        nc.scalar.activation(dst, src, Relu,
                                             bias=b1_t[:, m:m + 1])

        # ---- conv2 (3x3, 256->256, pad 1) + bias + relu -> out2 ----------
        # Per np-half: 8 open groups, contraction tap outer (each tap is a
        # DoubleRow pair over the two input channel blocks). The moving
        # operand is the shifted window over the interleaved padded field:
        # rows 2*dy..2*dy+27, cols dx..dx+13.
        for half in range(2):
            nps = [half * 4 + j for j in range(4)]
            grp = {}
            for np_ in nps:
                for m in range(KB2):
                    grp[(np_, m)] = psp.tile([P, NF], F32,
                                             name=f"ps2_{np_}_{m}", tag="ps")
            for t in range(9):
                dy, dx = t // 3, t % 3
                for m in range(KB2):
                    for np_ in nps:
                        rhs = out1v[:, :, np_, 2 * dy:2 * dy + 28,
                                    dx:dx + 14]
                        nc.tensor.matmul(
                            grp[(np_, m)][:]
                            .rearrange("p (j c) -> p j c", j=28),
                            w2v[:, t, :, m * P:(m + 1) * P],
                            rhs,
                            start=(t == 0), stop=(t == 8),
                            perf_mode=DR,
                        )
            for np_ in nps:
                for m in range(KB2):
                    dst = out2v[:, m, np_ * NF:(np_ + 1) * NF]
                    src = grp[(np_, m)][:]
                    if np_ % 2 == 1:
                        nc.vector.tensor_scalar(dst, src, b2_t[:, m:m + 1],
                                                0.0, Alu.add, Alu.max)
                    else:
                        nc.scalar.activation(dst, src, Relu,
                                             bias=b2_t[:, m:m + 1])

        # ---- conv3 (1x1, 256->1024) + residual + relu -> y --------------
        # Per m: 8 open groups (npairs). Each group is one DoubleRow matmul
        # (K=256) plus a bf16 identity matmul (weights s3*I) that adds
        # s3 * (x + b3) into PSUM; eviction is relu(psum)/s3, one op.
        inv_s3 = 1.0 / S3
        for m in range(MB3):
            grp = {}
            for np_ in range(NPAIRS):
                grp[np_] = psp.tile([P, NF], F32, name=f"ps3_{np_}", tag="ps")
            for np_ in range(NPAIRS):
                nc.tensor.matmul(
                    grp[np_][:],
                    w3v[:, :, m * P:(m + 1) * P],
                    out2v[:, :, np_ * NF:(np_ + 1) * NF],
                    start=True, stop=False,
                    perf_mode=DR,
                )
            for np_ in range(NPAIRS):
                nc.tensor.matmul(
                    grp[np_][:], id_t[:],
                    xrv[:, m, np_ * NF:(np_ + 1) * NF],
                    start=False, stop=True,
                )
            ystage = evp.tile([P, NLOC * HW], BF16, name="ystage",
                              tag="ystage", bufs=3)
            for np_ in range(NPAIRS):
                dst = ystage[:, np_ * NF:(np_ + 1) * NF]
                if np_ % 2 == 1:
                    nc.vector.tensor_scalar(dst, grp[np_][:], 0.0, inv_s3,
                                            Alu.max, Alu.mult)
                else:
                    nc.scalar.activation(dst, grp[np_][:], Relu,
                                         bias=0.0, scale=inv_s3)
            nchunk = 4 if m == MB3 - 1 else 2
            CNF = NLOC * HW // nchunk
            for c in range(nchunk):
                nc.sync.dma_start(y_d.ap()[m][:, c * CNF:(c + 1) * CNF],
                                  ystage[:, c * CNF:(c + 1) * CNF])


def _prep(x, w1, g1, b1, m1, v1, w2, g2, b2, m2, v2, w3, g3, b3, m3, v3):
    """Host-side: fold BN, scale + quantize weights to fp8, arrange SBUF
    images, shard + interleave x."""
    def fold(w, g, b, m, v):
        scale = (g.astype(np.float64) / np.sqrt(v.astype(np.float64) + EPS))
        bias = b.astype(np.float64) - m.astype(np.float64) * scale
        wf = w.astype(np.float64) * scale.reshape(-1, *([1] * (w.ndim - 1)))
        return wf.astype(np.float32), bias.astype(np.float32)

    w1f, bias1 = fold(w1, g1, b1, m1, v1)   # [256,1024,1,1]
    w2f, bias2 = fold(w2, g2, b2, m2, v2)   # [256,256,3,3]
    w3f, bias3 = fold(w3, g3, b3, m3, v3)   # [1024,256,1,1]

    f8 = ml_dtypes.float8_e4m3
    bf = ml_dtypes.bfloat16

    def q8(a):
        return np.clip(a, -240.0, 240.0).astype(f8)

    # w1 DoubleRow image [p, (kp i m)]: [p, kp, i, m] = w1f[m, (2kp+i)*128+p]
    w1t = np.ascontiguousarray(
        (w1f[:, :, 0, 0] * S1).T.reshape(KP1, 2, P, WIDTH)
        .transpose(2, 0, 1, 3).reshape(P, KP1 * 2 * WIDTH))
    # w2 image [p, (t i m)]: t = dy*3+dx, i = input block
    w2t = np.ascontiguousarray(
        (w2f * (S2 / S1)).transpose(2, 3, 1, 0)
        .reshape(9, KB2, P, WIDTH).transpose(2, 0, 1, 3)
        .reshape(P, 9 * KB2 * WIDTH))
    # w3 image [p, (i m)]
    w3t = np.ascontiguousarray(
        (w3f[:, :, 0, 0] * (S3 / S2)).T.reshape(KB2, P, C_OUT)
        .transpose(1, 0, 2).reshape(P, KB2 * C_OUT))

    b1h = (bias1 * S1).reshape(KB2, P).T                  # [P, 2]
    b2h = (bias2 * S2).reshape(KB2, P).T                  # [P, 2]
    ball = np.ascontiguousarray(
        np.concatenate([b1h, b2h], axis=1), dtype=np.float32)

    # x -> per-core [KB1, P, NLOC*HW] with columns (pair, j=2r+i, c):
    # [core, pair, i, kb, p, r, c] -> [core, kb, p, pair, r, i, c]
    xs = (x.reshape(NCORES, NPAIRS, 2, KB1, P, 14, 14)
          .transpose(0, 3, 4, 1, 5, 2, 6)
          .reshape(NCORES, KB1, P, NLOC * HW))
    xq = q8(xs)
    # residual: x + conv3 bias per channel, bf16
    xrf = x + bias3[None, :, None, None]
    xr = (xrf.reshape(NCORES, NPAIRS, 2, KB1, P, 14, 14)
          .transpose(0, 3, 4, 1, 5, 2, 6)
          .reshape(NCORES, KB1, P, NLOC * HW)).astype(bf)

    ident = (np.eye(P, dtype=np.float32) * S3).astype(bf)

    common = {"w1t": q8(w1t), "w2t": q8(w2t), "w3t": q8(w3t),
              "biases": ball, "ident": ident}
    in_maps = [dict(common,
                    xq=np.ascontiguousarray(xq[i]),
                    xr=np.ascontiguousarray(xr[i]))
               for i in range(NCORES)]
    return in_maps


def kernel(**inputs):
    inputs = {k: np.asarray(v) for k, v in inputs.items()}
    in_maps = _prep(**inputs)
    nc = _build()
    res = run_bass_kernel_spmd(nc, in_maps, core_ids=list(range(NCORES)))

    y = np.empty((NCORES * NLOC, C_OUT, 14, 14), dtype=np.float32)
    for i in range(NCORES):
        r = np.asarray(res.results[i]["y"], dtype=np.float32)  # [MB3,P,N*HW]
        # columns are (pair, j=2r+i, c): [m, p, pair, r, i, c]
        r = (r.reshape(MB3, P, NPAIRS, 14, 2, 14)
             .transpose(2, 4, 0, 1, 3, 5)
             .reshape(NLOC, C_OUT, 14, 14))
        y[i * NLOC:(i + 1) * NLOC] = r
    return y


# revision 6
# speedup vs baseline: 1.4548x; 1.0258x over previous
"""Trainium2 Bass kernel for a ResNet Bottleneck block (inference).

Reference computation (NCHW, N=128, Cin=Cout=1024, width=256, H=W=14):
    out = relu(bn1(conv1x1(x, w1)))          # 1024 -> 256
    out = relu(bn2(conv3x3(out, w2, pad=1))) # 256 -> 256
    out = bn3(conv1x1(out, w3))              # 256 -> 1024
    y   = relu(out + x)

Strategy:
- Data-parallel: batch 128 sharded as 16 images per NeuronCore (8 cores),
  conv/BN params replicated. One NEFF, SPMD via run_bass_kernel_spmd.
- BN folded on host into per-channel weight scale + bias.
- All convs run in fp8-e4m3 with MatmulPerfMode.DoubleRow: each matmul
  contracts K=256 (two 128-channel blocks stacked in AP dim 1) at double
  the bf16 MAC rate. Weights/activations are scaled host-side
  (s1=16, s2=64, s3=256) to sit in e4m3's healthy range; the scale is
  unwound for free: relu(s*a) = s*relu(a), so each conv's input scale is
  folded into the next conv's weights, and the final 1/s3 rides the
  eviction op.
- fp32 PSUM accumulation. conv3's residual is added in PSUM by a bf16
  identity-weight matmul (weights = s3*I) on the bf16 x tiles, so conv3
  eviction is a single relu-and-scale op per group. conv3's BN bias is
  folded into the residual tiles host-side (x + b3).
- Per-image-pair layouts use row-interleaved fields (j = 2*row + img) so
  the 3x3 conv's DoubleRow moving operand is a 4-dim AP
  [p, kpair, 28 interleaved rows, 14 cols] over a zero-padded 32x16
  field.
- PSUM groups are allocated as 2-bank pair tiles [P, 1024] so evictions
  process two groups per DVE/ACT op (halves op count + semaphores).
"""

import sys

if "/opt/trn_rl_repo" not in sys.path:
    sys.path.insert(0, "/opt/trn_rl_repo")

import numpy as np
import ml_dtypes

import concourse.bass as bass
import concourse.bacc as bacc
import concourse.tile as tile
from concourse import mybir
from concourse.bass_utils import run_bass_kernel_spmd

EPS = 1e-5
NCORES = 8
NLOC = 16          # images per core
C_IN = 1024
WIDTH = 256
C_OUT = 1024
HW = 196           # 14*14
P = 128
KB1 = C_IN // P    # 8 input channel blocks
KP1 = KB1 // 2     # 4 DoubleRow channel-block pairs for conv1
KB2 = WIDTH // P   # 2 channel blocks for conv2/conv3 input
MB3 = C_OUT // P   # 8 output channel blocks for conv3
NPAIRS = NLOC // 2  # 8 image pairs; N=392 per matmul
NF = 2 * HW        # 392
FLD = 512          # padded interleaved pair-field: 32 rows x 16 cols
BANK = 512         # PSUM bank, fp32 elements per partition

S1, S2, S3 = 16.0, 64.0, 256.0

F8 = mybir.dt.float8e4
BF16 = mybir.dt.bfloat16
F32 = mybir.dt.float32
Relu = mybir.ActivationFunctionType.Relu
DR = mybir.MatmulPerfMode.DoubleRow

_cached = {}


def _build():
    """Build + compile the SPMD NEFF (one core's program). Cached."""
    if "nc" in _cached:
        return _cached["nc"]

    nc = bacc.Bacc("TRN2", target_bir_lowering=False, debug=False,
                   num_devices=NCORES)

    xq_d = nc.dram_tensor("xq", [KB1, P, NLOC * HW], F8, kind="ExternalInput")
    xr_d = nc.dram_tensor("xr", [KB1, P, NLOC * HW], BF16,
                          kind="ExternalInput")
    w1_d = nc.dram_tensor("w1t", [P, KP1 * 2 * WIDTH], F8,
                          kind="ExternalInput")
    w2_d = nc.dram_tensor("w2t", [P, 9 * KB2 * WIDTH], F8,
                          kind="ExternalInput")
    w3_d = nc.dram_tensor("w3t", [P, KB2 * C_OUT], F8, kind="ExternalInput")
    b_d = nc.dram_tensor("biases", [P, 2 * KB2], F32, kind="ExternalInput")
    id_d = nc.dram_tensor("ident", [P, P], BF16, kind="ExternalInput")
    y_d = nc.dram_tensor("y", [MB3, P, NLOC * HW], BF16, kind="ExternalOutput")

    with tile.TileContext(nc) as tc:
        _emit(tc, nc, xq_d, xr_d, w1_d, w2_d, w3_d, b_d, id_d, y_d)

    nc.compile()
    _cached["nc"] = nc
    return nc


def _emit(tc, nc, xq_d, xr_d, w1_d, w2_d, w3_d, b_d, id_d, y_d):
    """PE-density-oriented emission.

    - Every phase runs its contraction loop OUTER over 8 concurrently-open
      PSUM groups (8 banks via 4 two-bank pair tiles), group index
      innermost, so consecutive matmuls target different banks and
      pipeline at the issue rate (~165ns for N=392 fp8 DoubleRow).
    - Startup: the PE clock needs ~3us of continuous activity to reach
      2.4 GHz, so warm-up matmuls are gated only on a scratch memset that
      is the FIRST gpsimd instruction (before any DMA issue). The xq
      stream gets exclusive DMA bandwidth until it completes; xr/w2/w3
      are chained behind it.
    - Evictions alternate DVE/ACT, one op per PSUM-bank pair.
    """
    import contextlib

    Alu = mybir.AluOpType

    with contextlib.ExitStack() as ctx:
        const = ctx.enter_context(tc.tile_pool(name="const", bufs=1))
        xpool = ctx.enter_context(tc.tile_pool(name="xpool", bufs=1))
        opool = ctx.enter_context(tc.tile_pool(name="opool", bufs=1))
        psp = ctx.enter_context(tc.tile_pool(name="psp", bufs=4, space="PSUM"))
        evp = ctx.enter_context(tc.tile_pool(name="evp", bufs=2))

        from concourse.tile import add_dep_helper

        # PE warm-up first: scratch memset is gpsimd's first instruction,
        # so the dummy matmuls start right after the framework preamble
        # and keep the PE busy (ramping its clock) while x loads.
        scratch = const.tile([P, 512], BF16, name="scratch", tag="scratch")
        nc.gpsimd.memset(scratch[:], 0.0)
        warm_ps = psp.tile([P, 2 * BANK], F32, name="warm_ps", tag="ps")
        for _ in range(8):
            nc.tensor.matmul(warm_ps[:, 0:512], scratch[:, 0:P], scratch[:],
                             start=True, stop=True)

        # ---- Loads: xq gets exclusive early bandwidth ---------------------
        w1sb = const.tile([P, KP1 * 2 * WIDTH], F8, name="w1sb", tag="w1sb")
        nc.scalar.dma_start(w1sb[:], w1_d.ap())
        w1v = w1sb[:].rearrange("p (j i c) -> p j i c", j=KP1, i=2)

        ball = const.tile([P, 2 * KB2], F32, name="ball", tag="ball")
        nc.scalar.dma_start(ball[:], b_d.ap())
        b1_t = ball[:, 0:KB2]
        b2_t = ball[:, KB2:2 * KB2]

        xq = xpool.tile([P, KB1 * NLOC * HW], F8, name="xq", tag="xq")
        xqv = xq[:].rearrange("p (k n) -> p k n", k=KB1)
        xq_dmas = []
        for j in range(KB1 // 2):
            dst = xqv[:, 2 * j:2 * j + 2, :]
            s = xq_d.ap()[2 * j:2 * j + 2].rearrange("k p c -> p k c")
            i = nc.sync.dma_start(dst, s)
            n = len(xq_dmas)
            if n >= 2:
                add_dep_helper(i.ins, xq_dmas[n - 2], reason="xq load pacing")
            xq_dmas.append(i.ins)

        w2sb = const.tile([P, 9 * KB2 * WIDTH], F8, name="w2sb", tag="w2sb")
        i = nc.gpsimd.dma_start(w2sb[:], w2_d.ap())
        add_dep_helper(i.ins, xq_dmas[2], reason="w2 behind xq bulk")
        w2v = w2sb[:].rearrange("p (t i c) -> p t i c", t=9, i=2)

        # residual x (bf16, conv3 bias folded in), behind xq
        xr = xpool.tile([P, KB1 * NLOC * HW], BF16, name="xr", tag="xr")
        xrv = xr[:].rearrange("p (k n) -> p k n", k=KB1)
        xr_dmas = []
        for j in range(KB1 // 2):
            dst = xrv[:, 2 * j:2 * j + 2, :]
            s = xr_d.ap()[2 * j:2 * j + 2].rearrange("k p c -> p k c")
            i = nc.gpsimd.dma_start(dst, s)
            n = len(xr_dmas)
            if n < 2:
                add_dep_helper(i.ins, xq_dmas[3], reason="xr behind xq")
            else:
                add_dep_helper(i.ins, xr_dmas[n - 2], reason="xr load pacing")
            xr_dmas.append(i.ins)

        w3sb = const.tile([P, KB2 * C_OUT], F8, name="w3sb", tag="w3sb")
        i = nc.gpsimd.dma_start(w3sb[:], w3_d.ap())
        add_dep_helper(i.ins, xr_dmas[1], reason="w3 behind xr")
        w3v = w3sb[:].rearrange("p (i c) -> p i c", i=2)

        id_t = const.tile([P, P], BF16, name="id_t", tag="id_t")
        i = nc.gpsimd.dma_start(id_t[:], id_d.ap())
        add_dep_helper(i.ins, xr_dmas[1], reason="ident behind xr")

        # conv1 output: zero-padded row-interleaved pair fields, fp8.
        # Per image pair a 32x16 field (j = 2*padrow + img), payload at
        # j in 2..29, cols 1..14. Layout [P, KB2 * NPAIRS * FLD].
        # Only the pad cells are zeroed (3 small memsets, not the full
        # field): top/bottom pad rows and the left/right pad columns.
        out1 = opool.tile([P, KB2 * NPAIRS * FLD], F8, name="out1",
                          tag="out1")
        kq = KB2 * NPAIRS  # 16 fields, stride FLD
        fv = out1[:].rearrange("p (f j c) -> p f j c", f=kq, j=32, c=16)
        nc.vector.memset(fv[:, :, 0:2, :], 0.0)      # top pad rows j=0,1
        nc.vector.memset(fv[:, :, 30:32, :], 0.0)    # bottom pad rows
        nc.vector.memset(fv[:, :, 2:30, 0:1], 0.0)   # left pad col
        nc.vector.memset(fv[:, :, 2:30, 15:16], 0.0)  # right pad col
        out1v = out1[:].rearrange("p (k q j c) -> p k q j c",
                                  k=KB2, q=NPAIRS, j=32, c=16)

        out2 = opool.tile([P, KB2 * NLOC * HW], F8, name="out2", tag="out2")
        out2v = out2[:].rearrange("p (k n) -> p k n", k=KB2)

        def pair_src(pt, shape=None):
            """[p, 2, 392] strided view over a 2-bank psum pair tile."""
            v = pt[:].rearrange("p (g s) -> p g s", g=2)[:, :, 0:NF]
            if shape == "jc":
                v = v.rearrange("p g (j c) -> p g j c", j=28)
            return v

        # ---- conv1 (1x1, 1024->256) + bias + relu -> padded out1 --------
        # Per np-half: 8 open groups as 4 pair tiles, pairing the two
        # output channel blocks of one np; contraction kp outer.
        for half in range(2):
            nps = [half * 4 + j for j in range(4)]
            pt = {np_: psp.tile([P, 2 * BANK], F32, name=f"ps1_{np_}",
                                tag="ps") for np_ in nps}
            for kp in range(KP1):
                for m in range(KB2):
                    for np_ in nps:
                        nc.tensor.matmul(
                            pt[np_][:, m * BANK:m * BANK + NF],
                            w1v[:, kp, :, m * P:(m + 1) * P],
                            xqv[:, 2 * kp:2 * kp + 2,
                                np_ * NF:(np_ + 1) * NF],
                            start=(kp == 0), stop=(kp == KP1 - 1),
                            perf_mode=DR,
                        )
            for np_ in nps:
                for m in range(KB2):
                    dst = out1v[:, m, np_, 2:30, 1:15]
                    src = (pt[np_][:, m * BANK:m * BANK + NF]
                           .rearrange("p (j c) -> p j c", j=28))
                    if np_ % 2 == 1:
                        nc.vector.tensor_scalar(dst, src, b1_t[:, m:m + 1],
                                                0.0, Alu.add, Alu.max)
                    else:
                        nc.scalar.activation(dst, src, Relu,
                                             bias=b1_t[:, m:m + 1])

        # ---- conv2 (3x3, 256->256, pad 1) + bias + relu -> out2 ----------
        # Per np-half: 8 open groups, contraction tap outer (each tap is a
        # DoubleRow pair over the two input channel blocks). The moving
        # operand is the shifted window over the interleaved padded field:
        # rows 2*dy..2*dy+27, cols dx..dx+13.
        for half in range(2):
            nps = [half * 4 + j for j in range(4)]
            pt = {np_: psp.tile([P, 2 * BANK], F32, name=f"ps2_{np_}",
                                tag="ps") for np_ in nps}
            for t in range(9):
                dy, dx = t // 3, t % 3
                for m in range(KB2):
                    for np_ in nps:
                        rhs = out1v[:, :, np_, 2 * dy:2 * dy + 28,
                                    dx:dx + 14]
                        nc.tensor.matmul(
                            pt[np_][:, m * BANK:m * BANK + NF]
                            .rearrange("p (j c) -> p j c", j=28),
                            w2v[:, t, :, m * P:(m + 1) * P],
                            rhs,
                            start=(t == 0), stop=(t == 8),
                            perf_mode=DR,
                        )
            for np_ in nps:
                for m in range(KB2):
                    dst = out2v[:, m, np_ * NF:(np_ + 1) * NF]
                    src = pt[np_][:, m * BANK:m * BANK + NF]
                    if np_ % 2 == 1:
                        nc.vector.tensor_scalar(dst, src, b2_t[:, m:m + 1],
                                                0.0, Alu.add, Alu.max)
                    else:
                        nc.scalar.activation(dst, src, Relu,
                                             bias=b2_t[:, m:m + 1])

        # ---- conv3 (1x1, 256->1024) + residual + relu -> y --------------
        # Per m: 8 open groups as 4 pair tiles. Each group is one DoubleRow
        # matmul (K=256) plus a bf16 identity matmul (weights s3*I) adding
        # s3 * (x + b3) into PSUM; eviction is relu(psum)/s3 per pair.
        inv_s3 = 1.0 / S3
        for m in range(MB3):
            pt = {g: psp.tile([P, 2 * BANK], F32, name=f"ps3_{g}", tag="ps")
                  for g in range(4)}
            for np_ in range(NPAIRS):
                nc.tensor.matmul(
                    pt[np_ // 2][:, (np_ % 2) * BANK:(np_ % 2) * BANK + NF],
                    w3v[:, :, m * P:(m + 1) * P],
                    out2v[:, :, np_ * NF:(np_ + 1) * NF],
                    start=True, stop=False,
                    perf_mode=DR,
                )
            for np_ in range(NPAIRS):
                nc.tensor.matmul(
                    pt[np_ // 2][:, (np_ % 2) * BANK:(np_ % 2) * BANK + NF],
                    id_t[:],
                    xrv[:, m, np_ * NF:(np_ + 1) * NF],
                    start=False, stop=True,
                )
            ystage = evp.tile([P, NLOC * HW], BF16, name="ystage",
                              tag="ystage", bufs=3)
            for g in range(4):
                dst = (ystage[:, 2 * g * NF:(2 * g + 2) * NF]
                       .rearrange("p (g n) -> p g n", g=2))
                src = pair_src(pt[g])
                if (g + m) % 2 == 1:
                    nc.vector.tensor_scalar(dst, src, 0.0, inv_s3,
                                            Alu.max, Alu.mult)
                else:
                    nc.scalar.activation(dst, src, Relu,
                                         bias=0.0, scale=inv_s3)
            nchunk = 4 if m == MB3 - 1 else 2
            CNF = NLOC * HW // nchunk
            for c in range(nchunk):
                nc.sync.dma_start(y_d.ap()[m][:, c * CNF:(c + 1) * CNF],
                                  ystage[:, c * CNF:(c + 1) * CNF])


def _prep(x, w1, g1, b1, m1, v1, w2, g2, b2, m2, v2, w3, g3, b3, m3, v3):
    """Host-side: fold BN, scale + quantize weights to fp8, arrange SBUF
    images, shard + interleave x."""
    def fold(w, g, b, m, v):
        scale = (g.astype(np.float64) / np.sqrt(v.astype(np.float64) + EPS))
        bias = b.astype(np.float64) - m.astype(np.float64) * scale
        wf = w.astype(np.float64) * scale.reshape(-1, *([1] * (w.ndim - 1)))
        return wf.astype(np.float32), bias.astype(np.float32)

    w1f, bias1 = fold(w1, g1, b1, m1, v1)   # [256,1024,1,1]
    w2f, bias2 = fold(w2, g2, b2, m2, v2)   # [256,256,3,3]
    w3f, bias3 = fold(w3, g3, b3, m3, v3)   # [1024,256,1,1]

    f8 = ml_dtypes.float8_e4m3
    bf = ml_dtypes.bfloat16

    def q8(a):
        return np.clip(a, -240.0, 240.0).astype(f8)

    # w1 DoubleRow image [p, (kp i m)]: [p, kp, i, m] = w1f[m, (2kp+i)*128+p]
    w1t = np.ascontiguousarray(
        (w1f[:, :, 0, 0] * S1).T.reshape(KP1, 2, P, WIDTH)
        .transpose(2, 0, 1, 3).reshape(P, KP1 * 2 * WIDTH))
    # w2 image [p, (t i m)]: t = dy*3+dx, i = input block
    w2t = np.ascontiguousarray(
        (w2f * (S2 / S1)).transpose(2, 3, 1, 0)
        .reshape(9, KB2, P, WIDTH).transpose(2, 0, 1, 3)
        .reshape(P, 9 * KB2 * WIDTH))
    # w3 image [p, (i m)]
    w3t = np.ascontiguousarray(
        (w3f[:, :, 0, 0] * (S3 / S2)).T.reshape(KB2, P, C_OUT)
        .transpose(1, 0, 2).reshape(P, KB2 * C_OUT))

    b1h = (bias1 * S1).reshape(KB2, P).T                  # [P, 2]
    b2h = (bias2 * S2).reshape(KB2, P).T                  # [P, 2]
    ball = np.ascontiguousarray(
        np.concatenate([b1h, b2h], axis=1), dtype=np.float32)

    # x -> per-core [KB1, P, NLOC*HW] with columns (pair, j=2r+i, c):
    # [core, pair, i, kb, p, r, c] -> [core, kb, p, pair, r, i, c]
    xs = (x.reshape(NCORES, NPAIRS, 2, KB1, P, 14, 14)
          .transpose(0, 3, 4, 1, 5, 2, 6)
          .reshape(NCORES, KB1, P, NLOC * HW))
    xq = q8(xs)
    # residual: x + conv3 bias per channel, bf16
    xrf = x + bias3[None, :, None, None]
    xr = (xrf.reshape(NCORES, NPAIRS, 2, KB1, P, 14, 14)
          .transpose(0, 3, 4, 1, 5, 2, 6)
          .reshape(NCORES, KB1, P, NLOC * HW)).astype(bf)

    ident = (np.eye(P, dtype=np.float32) * S3).astype(bf)

    common = {"w1t": q8(w1t), "w2t": q8(w2t), "w3t": q8(w3t),
              "biases": ball, "ident": ident}
    in_maps = [dict(common,
                    xq=np.ascontiguousarray(xq[i]),
                    xr=np.ascontiguousarray(xr[i]))
               for i in range(NCORES)]
    return in_maps


def kernel(**inputs):
    inputs = {k: np.asarray(v) for k, v in inputs.items()}
    in_maps = _prep(**inputs)
    nc = _build()
    res = run_bass_kernel_spmd(nc, in_maps, core_ids=list(range(NCORES)))

    y = np.empty((NCORES * NLOC, C_OUT, 14, 14), dtype=np.float32)
    for i in range(NCORES):
        r = np.asarray(res.results[i]["y"], dtype=np.float32)  # [MB3,P,N*HW]
        # columns are (pair, j=2r+i, c): [m, p, pair, r, i, c]
        r = (r.reshape(MB3, P, NPAIRS, 14, 2, 14)
             .transpose(2, 4, 0, 1, 3, 5)
             .reshape(NLOC, C_OUT, 14, 14))
        y[i * NLOC:(i + 1) * NLOC] = r
    return y


# revision 15
# speedup vs baseline: 1.4654x; 1.0073x over previous
"""Trainium2 Bass kernel for a ResNet Bottleneck block (inference).

Reference computation (NCHW, N=128, Cin=Cout=1024, width=256, H=W=14):
    out = relu(bn1(conv1x1(x, w1)))          # 1024 -> 256
    out = relu(bn2(conv3x3(out, w2, pad=1))) # 256 -> 256
    out = bn3(conv1x1(out, w3))              # 256 -> 1024
    y   = relu(out + x)

Strategy:
- Data-parallel: batch 128 sharded as 16 images per NeuronCore (8 cores),
  conv/BN params replicated. One NEFF, SPMD via run_bass_kernel_spmd.
- BN folded on host into per-channel weight scale + bias.
- All convs run in fp8-e4m3 with MatmulPerfMode.DoubleRow: each matmul
  contracts K=256 (two 128-channel blocks stacked in AP dim 1) at double
  the bf16 MAC rate. Weights/activations are scaled host-side
  (s1=16, s2=64, s3=256) to sit in e4m3's healthy range; the scale is
  unwound for free: relu(s*a) = s*relu(a), so each conv's input scale is
  folded into the next conv's weights, and the final 1/s3 rides the
  eviction op.
- fp32 PSUM accumulation. conv3's residual is added in PSUM by a bf16
  identity-weight matmul (weights = s3*I) on the bf16 x tiles, so conv3
  eviction is a single relu-and-scale op per group. conv3's BN bias is
  folded into the residual tiles host-side (x + b3).
- Per-image-pair layouts use row-interleaved fields (j = 2*row + img) so
  the 3x3 conv's DoubleRow moving operand is a 4-dim AP
  [p, kpair, 28 interleaved rows, 14 cols] over a zero-padded 32x16
  field.
- PSUM groups are allocated as 2-bank pair tiles [P, 1024] so evictions
  process two groups per DVE/ACT op (halves op count + semaphores).
"""

import sys

if "/opt/trn_rl_repo" not in sys.path:
    sys.path.insert(0, "/opt/trn_rl_repo")

import numpy as np
import ml_dtypes

import concourse.bass as bass
import concourse.bacc as bacc
import concourse.tile as tile
from concourse import mybir
from concourse.bass_utils import run_bass_kernel_spmd

EPS = 1e-5
NCORES = 8
NLOC = 16          # images per core
C_IN = 1024
WIDTH = 256
C_OUT = 1024
HW = 196           # 14*14
P = 128
KB1 = C_IN // P    # 8 input channel blocks
KP1 = KB1 // 2     # 4 DoubleRow channel-block pairs for conv1
KB2 = WIDTH // P   # 2 channel blocks for conv2/conv3 input
MB3 = C_OUT // P   # 8 output channel blocks for conv3
NPAIRS = NLOC // 2  # 8 image pairs; N=392 per matmul
NF = 2 * HW        # 392
FLD = 512          # padded interleaved pair-field: 32 rows x 16 cols
BANK = 512         # PSUM bank, fp32 elements per partition

S1, S2, S3 = 16.0, 64.0, 256.0

F8 = mybir.dt.float8e4
BF16 = mybir.dt.bfloat16
F32 = mybir.dt.float32
Relu = mybir.ActivationFunctionType.Relu
DR = mybir.MatmulPerfMode.DoubleRow

_cached = {}


def _build():
    """Build + compile the SPMD NEFF (one core's program). Cached."""
    if "nc" in _cached:
        return _cached["nc"]

    nc = bacc.Bacc("TRN2", target_bir_lowering=False, debug=False,
                   num_devices=NCORES)

    # x tensors are partition-major in DRAM: per partition one long
    # contiguous run per DMA chunk (best descriptor efficiency)
    xq_d = nc.dram_tensor("xq", [P, KB1 * NLOC * HW], F8,
                          kind="ExternalInput")
    xr_d = nc.dram_tensor("xr", [P, KB1 * NLOC * HW], BF16,
                          kind="ExternalInput")
    w1_d = nc.dram_tensor("w1t", [P, KP1 * 2 * WIDTH], F8,
                          kind="ExternalInput")
    w2_d = nc.dram_tensor("w2t", [P, 9 * KB2 * WIDTH], F8,
                          kind="ExternalInput")
    w3_d = nc.dram_tensor("w3t", [P, KB2 * C_OUT], F8, kind="ExternalInput")
    b_d = nc.dram_tensor("biases", [P, 2 * KB2], F32, kind="ExternalInput")
    id_d = nc.dram_tensor("ident", [P, P], BF16, kind="ExternalInput")
    y_d = nc.dram_tensor("y", [MB3, P, NLOC * HW], BF16, kind="ExternalOutput")

    with tile.TileContext(nc) as tc:
        _emit(tc, nc, xq_d, xr_d, w1_d, w2_d, w3_d, b_d, id_d, y_d)

    nc.compile()
    _cached["nc"] = nc
    return nc


def _emit(tc, nc, xq_d, xr_d, w1_d, w2_d, w3_d, b_d, id_d, y_d):
    """PE-density-oriented emission.

    - Every phase runs its contraction loop OUTER over 8 concurrently-open
      PSUM groups (8 banks via 4 two-bank pair tiles), group index
      innermost, so consecutive matmuls target different banks and
      pipeline at the issue rate (~165ns for N=392 fp8 DoubleRow).
    - Startup: the PE clock needs ~3us of continuous activity to reach
      2.4 GHz, so warm-up matmuls are gated only on a scratch memset that
      is the FIRST gpsimd instruction (before any DMA issue). The xq
      stream gets exclusive DMA bandwidth until it completes; xr/w2/w3
      are chained behind it.
    - Evictions alternate DVE/ACT, one op per PSUM-bank pair.
    """
    import contextlib

    Alu = mybir.AluOpType

    with contextlib.ExitStack() as ctx:
        const = ctx.enter_context(tc.tile_pool(name="const", bufs=1))
        xpool = ctx.enter_context(tc.tile_pool(name="xpool", bufs=1))
        opool = ctx.enter_context(tc.tile_pool(name="opool", bufs=1))
        psp = ctx.enter_context(tc.tile_pool(name="psp", bufs=8, space="PSUM"))
        evp = ctx.enter_context(tc.tile_pool(name="evp", bufs=2))

        from concourse.tile import add_dep_helper

        # PE warm-up first: scratch memset is gpsimd's first instruction,
        # so the dummy matmuls start right after the framework preamble
        # and keep the PE busy (ramping its clock) while x loads.
        scratch = const.tile([P, 512], BF16, name="scratch", tag="scratch")
        nc.gpsimd.memset(scratch[:], 0.0)
        warm_ps = psp.tile([P, 512], F32, name="warm_ps", tag="ps")
        for _ in range(8):
            nc.tensor.matmul(warm_ps[:], scratch[:, 0:P], scratch[:],
                             start=True, stop=True)

        # ---- Loads: xq gets exclusive early bandwidth ---------------------
        w1sb = const.tile([P, KP1 * 2 * WIDTH], F8, name="w1sb", tag="w1sb")
        nc.scalar.dma_start(w1sb[:], w1_d.ap())
        w1v = w1sb[:].rearrange("p (j i c) -> p j i c", j=KP1, i=2)

        ball = const.tile([P, 2 * KB2], F32, name="ball", tag="ball")
        nc.scalar.dma_start(ball[:], b_d.ap())
        b1_t = ball[:, 0:KB2]
        b2_t = ball[:, KB2:2 * KB2]

        xq = xpool.tile([P, KB1 * NLOC * HW], F8, name="xq", tag="xq")
        xqv = xq[:].rearrange("p (k n) -> p k n", k=KB1)
        CH = 2 * NLOC * HW  # chunk: 2 channel blocks, contiguous both sides
        xq_dmas = []
        for j in range(KB1 // 2):
            i = nc.sync.dma_start(xq[:, j * CH:(j + 1) * CH],
                                  xq_d.ap()[:, j * CH:(j + 1) * CH])
            n = len(xq_dmas)
            if n >= 2:
                add_dep_helper(i.ins, xq_dmas[n - 2], reason="xq load pacing")
            xq_dmas.append(i.ins)

        w2sb = const.tile([P, 9 * KB2 * WIDTH], F8, name="w2sb", tag="w2sb")
        i = nc.gpsimd.dma_start(w2sb[:], w2_d.ap())
        add_dep_helper(i.ins, xq_dmas[2], reason="w2 behind xq bulk")
        w2v = w2sb[:].rearrange("p (t i c) -> p t i c", t=9, i=2)

        # residual x (bf16, conv3 bias folded in), behind xq
        xr = xpool.tile([P, KB1 * NLOC * HW], BF16, name="xr", tag="xr")
        xrv = xr[:].rearrange("p (k n) -> p k n", k=KB1)
        xr_dmas = []
        for j in range(KB1 // 2):
            i = nc.gpsimd.dma_start(xr[:, j * CH:(j + 1) * CH],
                                    xr_d.ap()[:, j * CH:(j + 1) * CH])
            n = len(xr_dmas)
            if n < 2:
                add_dep_helper(i.ins, xq_dmas[3], reason="xr behind xq")
            else:
                add_dep_helper(i.ins, xr_dmas[n - 2], reason="xr load pacing")
            xr_dmas.append(i.ins)

        w3sb = const.tile([P, KB2 * C_OUT], F8, name="w3sb", tag="w3sb")
        i = nc.gpsimd.dma_start(w3sb[:], w3_d.ap())
        add_dep_helper(i.ins, xr_dmas[1], reason="w3 behind xr")
        w3v = w3sb[:].rearrange("p (i c) -> p i c", i=2)

        id_t = const.tile([P, P], BF16, name="id_t", tag="id_t")
        i = nc.gpsimd.dma_start(id_t[:], id_d.ap())
        add_dep_helper(i.ins, xr_dmas[1], reason="ident behind xr")

        # conv1 output: zero-padded row-interleaved pair fields, fp8.
        # Per image pair a 32x16 field (j = 2*padrow + img), payload at
        # j in 2..29, cols 1..14. Layout [P, KB2 * NPAIRS * FLD].
        # Only the pad cells are zeroed (3 small memsets, not the full
        # field): top/bottom pad rows and the left/right pad columns.
        out1 = opool.tile([P, KB2 * NPAIRS * FLD], F8, name="out1",
                          tag="out1")
        kq = KB2 * NPAIRS  # 16 fields, stride FLD
        fv = out1[:].rearrange("p (f j c) -> p f j c", f=kq, j=32, c=16)
        nc.vector.memset(fv[:, :, 0:2, :], 0.0)      # top pad rows j=0,1
        nc.vector.memset(fv[:, :, 30:32, :], 0.0)    # bottom pad rows
        nc.vector.memset(fv[:, :, 2:30, 0:1], 0.0)   # left pad col
        nc.vector.memset(fv[:, :, 2:30, 15:16], 0.0)  # right pad col
        out1v = out1[:].rearrange("p (k q j c) -> p k q j c",
                                  k=KB2, q=NPAIRS, j=32, c=16)

        out2 = opool.tile([P, KB2 * NLOC * HW], F8, name="out2", tag="out2")
        out2v = out2[:].rearrange("p (k n) -> p k n", k=KB2)

        # ---- conv1 (1x1, 1024->256) + bias + relu -> padded out1 --------
        # Per np-half: 8 open groups (4 npairs x 2 m), contraction kp outer.
        for half in range(2):
            nps = [half * 4 + j for j in range(4)]
            grp = {}
            for np_ in nps:
                for m in range(KB2):
                    grp[(np_, m)] = psp.tile([P, NF], F32,
                                             name=f"ps1_{np_}_{m}", tag="ps")
            for kp in range(KP1):
                for m in range(KB2):
                    for np_ in nps:
                        nc.tensor.matmul(
                            grp[(np_, m)][:],
                            w1v[:, kp, :, m * P:(m + 1) * P],
                            xqv[:, 2 * kp:2 * kp + 2,
                                np_ * NF:(np_ + 1) * NF],
                            start=(kp == 0), stop=(kp == KP1 - 1),
                            perf_mode=DR,
                        )
            for np_ in nps:
                for m in range(KB2):
                    dst = out1v[:, m, np_, 2:30, 1:15]
                    src = (grp[(np_, m)][:]
                           .rearrange("p (j c) -> p j c", j=28))
                    if np_ % 2 == 1:
                        nc.vector.tensor_scalar(dst, src, b1_t[:, m:m + 1],
                                                0.0, Alu.add, Alu.max)
                    else:
                        nc.scalar.activation(dst, src, Relu,
                                             bias=b1_t[:, m:m + 1])

        # ---- conv2 (3x3, 256->256, pad 1) + bias + relu -> out2 ----------
        # Per np-half: 8 open groups, contraction tap outer (each tap is a
        # DoubleRow pair over the two input channel blocks). The moving
        # operand is the shifted window over the interleaved padded field:
        # rows 2*dy..2*dy+27, cols dx..dx+13.
        for half in range(2):
            nps = [half * 4 + j for j in range(4)]
            grp = {}
            for np_ in nps:
                for m in range(KB2):
                    grp[(np_, m)] = psp.tile([P, NF], F32,
                                             name=f"ps2_{np_}_{m}", tag="ps")
            for t in range(9):
                dy, dx = t // 3, t % 3
                for m in range(KB2):
                    for np_ in nps:
                        rhs = out1v[:, :, np_, 2 * dy:2 * dy + 28,
                                    dx:dx + 14]
                        nc.tensor.matmul(
                            grp[(np_, m)][:]
                            .rearrange("p (j c) -> p j c", j=28),
                            w2v[:, t, :, m * P:(m + 1) * P],
                            rhs,
                            start=(t == 0), stop=(t == 8),
                            perf_mode=DR,
                        )
            for np_ in nps:
                for m in range(KB2):
                    dst = out2v[:, m, np_ * NF:(np_ + 1) * NF]
                    src = grp[(np_, m)][:]
                    if np_ % 2 == 1:
                        nc.vector.tensor_scalar(dst, src, b2_t[:, m:m + 1],
                                                0.0, Alu.add, Alu.max)
                    else:
                        nc.scalar.activation(dst, src, Relu,
                                             bias=b2_t[:, m:m + 1])

        # ---- conv3 (1x1, 256->1024) + residual + relu -> y --------------
        # Per m: 8 open groups as 4 pair tiles. Each group is one DoubleRow
        # matmul (K=256) plus a bf16 identity matmul (weights s3*I) adding
        # s3 * (x + b3) into PSUM; eviction is relu(psum)/s3 per pair.
        inv_s3 = 1.0 / S3
        for m in range(MB3):
            grp = {}
            for np_ in range(NPAIRS):
                grp[np_] = psp.tile([P, NF], F32, name=f"ps3_{np_}", tag="ps")
            for np_ in range(NPAIRS):
                nc.tensor.matmul(
                    grp[np_][:],
                    w3v[:, :, m * P:(m + 1) * P],
                    out2v[:, :, np_ * NF:(np_ + 1) * NF],
                    start=True, stop=False,
                    perf_mode=DR,
                )
            for np_ in range(NPAIRS):
                nc.tensor.matmul(
                    grp[np_][:], id_t[:],
                    xrv[:, m, np_ * NF:(np_ + 1) * NF],
                    start=False, stop=True,
                )
            ystage = evp.tile([P, NLOC * HW], BF16, name="ystage",
                              tag="ystage", bufs=3)
            for np_ in range(NPAIRS):
                dst = ystage[:, np_ * NF:(np_ + 1) * NF]
                if (np_ + m) % 2 == 1:
                    nc.vector.tensor_scalar(dst, grp[np_][:], 0.0, inv_s3,
                                            Alu.max, Alu.mult)
                else:
                    nc.scalar.activation(dst, grp[np_][:], Relu,
                                         bias=0.0, scale=inv_s3)
            nchunk = 4 if m == MB3 - 1 else 2
            CNF = NLOC * HW // nchunk
            for c in range(nchunk):
                nc.sync.dma_start(y_d.ap()[m][:, c * CNF:(c + 1) * CNF],
                                  ystage[:, c * CNF:(c + 1) * CNF])


def _prep(x, w1, g1, b1, m1, v1, w2, g2, b2, m2, v2, w3, g3, b3, m3, v3):
    """Host-side: fold BN, scale + quantize weights to fp8, arrange SBUF
    images, shard + interleave x."""
    def fold(w, g, b, m, v):
        scale = (g.astype(np.float64) / np.sqrt(v.astype(np.float64) + EPS))
        bias = b.astype(np.float64) - m.astype(np.float64) * scale
        wf = w.astype(np.float64) * scale.reshape(-1, *([1] * (w.ndim - 1)))
        return wf.astype(np.float32), bias.astype(np.float32)

    w1f, bias1 = fold(w1, g1, b1, m1, v1)   # [256,1024,1,1]
    w2f, bias2 = fold(w2, g2, b2, m2, v2)   # [256,256,3,3]
    w3f, bias3 = fold(w3, g3, b3, m3, v3)   # [1024,256,1,1]

    f8 = ml_dtypes.float8_e4m3
    bf = ml_dtypes.bfloat16

    def q8(a):
        return np.clip(a, -240.0, 240.0).astype(f8)

    # w1 DoubleRow image [p, (kp i m)]: [p, kp, i, m] = w1f[m, (2kp+i)*128+p]
    w1t = np.ascontiguousarray(
        (w1f[:, :, 0, 0] * S1).T.reshape(KP1, 2, P, WIDTH)
        .transpose(2, 0, 1, 3).reshape(P, KP1 * 2 * WIDTH))
    # w2 image [p, (t i m)]: t = dy*3+dx, i = input block
    w2t = np.ascontiguousarray(
        (w2f * (S2 / S1)).transpose(2, 3, 1, 0)
        .reshape(9, KB2, P, WIDTH).transpose(2, 0, 1, 3)
        .reshape(P, 9 * KB2 * WIDTH))
    # w3 image [p, (i m)]
    w3t = np.ascontiguousarray(
        (w3f[:, :, 0, 0] * (S3 / S2)).T.reshape(KB2, P, C_OUT)
        .transpose(1, 0, 2).reshape(P, KB2 * C_OUT))

    b1h = (bias1 * S1).reshape(KB2, P).T                  # [P, 2]
    b2h = (bias2 * S2).reshape(KB2, P).T                  # [P, 2]
    ball = np.ascontiguousarray(
        np.concatenate([b1h, b2h], axis=1), dtype=np.float32)

    # x -> per-core partition-major [P, KB1*NLOC*HW], columns per k-block
    # ordered (pair, j=2r+i, c):
    # [core, pair, i, kb, p, r, c] -> [core, p, kb, pair, r, i, c]
    xs = (x.reshape(NCORES, NPAIRS, 2, KB1, P, 14, 14)
          .transpose(0, 4, 3, 1, 5, 2, 6)
          .reshape(NCORES, P, KB1 * NLOC * HW))
    xq = q8(xs)
    # residual: x + conv3 bias per channel, bf16
    xrf = x + bias3[None, :, None, None]
    xr = (xrf.reshape(NCORES, NPAIRS, 2, KB1, P, 14, 14)
          .transpose(0, 4, 3, 1, 5, 2, 6)
          .reshape(NCORES, P, KB1 * NLOC * HW)).astype(bf)

    ident = (np.eye(P, dtype=np.float32) * S3).astype(bf)

    common = {"w1t": q8(w1t), "w2t": q8(w2t), "w3t": q8(w3t),
              "biases": ball, "ident": ident}
    in_maps = [dict(common,
                    xq=np.ascontiguousarray(xq[i]),
                    xr=np.ascontiguousarray(xr[i]))
               for i in range(NCORES)]
    return in_maps


def kernel(**inputs):
    inputs = {k: np.asarray(v) for k, v in inputs.items()}
    in_maps = _prep(**inputs)
    nc = _build()
    res = run_bass_kernel_spmd(nc, in_maps, core_ids=list(range(NCORES)))

    y = np.empty((NCORES * NLOC, C_OUT, 14, 14), dtype=np.float32)
    for i in range(NCORES):
        r = np.asarray(res.results[i]["y"], dtype=np.float32)  # [MB3,P,N*HW]
        # columns are (pair, j=2r+i, c): [m, p, pair, r, i, c]
        r = (r.reshape(MB3, P, NPAIRS, 14, 2, 14)
             .transpose(2, 4, 0, 1, 3, 5)
             .reshape(NLOC, C_OUT, 14, 14))
        y[i * NLOC:(i + 1) * NLOC] = r
    return y


# revision 17
# speedup vs baseline: 1.5137x; 1.0330x over previous
"""Trainium2 Bass kernel for a ResNet Bottleneck block (inference).

Reference computation (NCHW, N=128, Cin=Cout=1024, width=256, H=W=14):
    out = relu(bn1(conv1x1(x, w1)))          # 1024 -> 256
    out = relu(bn2(conv3x3(out, w2, pad=1))) # 256 -> 256
    out = bn3(conv1x1(out, w3))              # 256 -> 1024
    y   = relu(out + x)

Strategy:
- Data-parallel: batch 128 sharded as 16 images per NeuronCore (8 cores),
  conv/BN params replicated. One NEFF, SPMD via run_bass_kernel_spmd.
- BN folded on host into per-channel weight scale + bias.
- All convs run in fp8-e4m3 with MatmulPerfMode.DoubleRow: each matmul
  contracts K=256 (two 128-channel blocks stacked in AP dim 1) at double
  the bf16 MAC rate. Weights/activations are scaled host-side
  (s1=16, s2=64, s3=256) to sit in e4m3's healthy range; the scale is
  unwound for free: relu(s*a) = s*relu(a), so each conv's input scale is
  folded into the next conv's weights, and the final 1/s3 rides the
  eviction op.
- fp32 PSUM accumulation. conv3's residual is added in PSUM by a bf16
  identity-weight matmul (weights = s3*I) on the bf16 x tiles, so conv3
  eviction is a single relu-and-scale op per group. conv3's BN bias is
  folded into the residual tiles host-side (x + b3).
- Per-image-pair layouts use row-interleaved fields (j = 2*row + img) so
  the 3x3 conv's DoubleRow moving operand is a 4-dim AP
  [p, kpair, 28 interleaved rows, 14 cols] over a zero-padded 32x16
  field.
- PSUM groups are allocated as 2-bank pair tiles [P, 1024] so evictions
  process two groups per DVE/ACT op (halves op count + semaphores).
"""

import sys

if "/opt/trn_rl_repo" not in sys.path:
    sys.path.insert(0, "/opt/trn_rl_repo")

import numpy as np
import ml_dtypes

import concourse.bass as bass
import concourse.bacc as bacc
import concourse.tile as tile
from concourse import mybir
from concourse.bass_utils import run_bass_kernel_spmd

EPS = 1e-5
NCORES = 8
NLOC = 16          # images per core
C_IN = 1024
WIDTH = 256
C_OUT = 1024
HW = 196           # 14*14
P = 128
KB1 = C_IN // P    # 8 input channel blocks
KP1 = KB1 // 2     # 4 DoubleRow channel-block pairs for conv1
KB2 = WIDTH // P   # 2 channel blocks for conv2/conv3 input
MB3 = C_OUT // P   # 8 output channel blocks for conv3
NPAIRS = NLOC // 2  # 8 image pairs; N=392 per matmul
NF = 2 * HW        # 392
FLD = 512          # padded interleaved pair-field: 32 rows x 16 cols
BANK = 512         # PSUM bank, fp32 elements per partition

S1, S2, S3 = 16.0, 64.0, 256.0

F8 = mybir.dt.float8e4
BF16 = mybir.dt.bfloat16
F32 = mybir.dt.float32
Relu = mybir.ActivationFunctionType.Relu
DR = mybir.MatmulPerfMode.DoubleRow

_cached = {}


def _build():
    """Build + compile the SPMD NEFF (one core's program). Cached."""
    if "nc" in _cached:
        return _cached["nc"]

    nc = bacc.Bacc("TRN2", target_bir_lowering=False, debug=False,
                   num_devices=NCORES)

    # x tensors are partition-major in DRAM: per partition one long
    # contiguous run per DMA chunk (best descriptor efficiency)
    xq_d = nc.dram_tensor("xq", [P, KB1 * NLOC * HW], F8,
                          kind="ExternalInput")
    xr_d = nc.dram_tensor("xr", [P, KB1 * NLOC * HW], BF16,
                          kind="ExternalInput")
    w1_d = nc.dram_tensor("w1t", [P, KP1 * 2 * WIDTH], F8,
                          kind="ExternalInput")
    w2_d = nc.dram_tensor("w2t", [P, 9 * KB2 * WIDTH], F8,
                          kind="ExternalInput")
    w3_d = nc.dram_tensor("w3t", [P, KB2 * C_OUT], F8, kind="ExternalInput")
    b_d = nc.dram_tensor("biases", [P, 2 * KB2], F32, kind="ExternalInput")
    id_d = nc.dram_tensor("ident", [P, P], BF16, kind="ExternalInput")
    y_d = nc.dram_tensor("y", [MB3, P, NLOC * HW], BF16, kind="ExternalOutput")

    with tile.TileContext(nc) as tc:
        _emit(tc, nc, xq_d, xr_d, w1_d, w2_d, w3_d, b_d, id_d, y_d)

    nc.compile()
    _cached["nc"] = nc
    return nc


def _emit(tc, nc, xq_d, xr_d, w1_d, w2_d, w3_d, b_d, id_d, y_d):
    """PE-density-oriented emission.

    - Every phase runs its contraction loop OUTER over 8 concurrently-open
      PSUM groups (8 banks via 4 two-bank pair tiles), group index
      innermost, so consecutive matmuls target different banks and
      pipeline at the issue rate (~165ns for N=392 fp8 DoubleRow).
    - Startup: the PE clock needs ~3us of continuous activity to reach
      2.4 GHz, so warm-up matmuls are gated only on a scratch memset that
      is the FIRST gpsimd instruction (before any DMA issue). The xq
      stream gets exclusive DMA bandwidth until it completes; xr/w2/w3
      are chained behind it.
    - Evictions alternate DVE/ACT, one op per PSUM-bank pair.
    """
    import contextlib

    Alu = mybir.AluOpType

    with contextlib.ExitStack() as ctx:
        const = ctx.enter_context(tc.tile_pool(name="const", bufs=1))
        xpool = ctx.enter_context(tc.tile_pool(name="xpool", bufs=1))
        opool = ctx.enter_context(tc.tile_pool(name="opool", bufs=1))
        psp = ctx.enter_context(tc.tile_pool(name="psp", bufs=8, space="PSUM"))
        evp = ctx.enter_context(tc.tile_pool(name="evp", bufs=2))

        from concourse.tile import add_dep_helper

        # PE warm-up first: scratch memset is gpsimd's first instruction,
        # so the dummy matmuls start right after the framework preamble
        # and keep the PE busy (ramping its clock) while x loads.
        # Warm-up across 6 different PSUM banks so the dummy matmuls
        # pipeline at issue rate (same-bank back-to-back matmuls serialize
        # at full round-trip latency and never ramp the clock).
        scratch = const.tile([P, 512], BF16, name="scratch", tag="scratch")
        nc.gpsimd.memset(scratch[:], 0.0)
        for w in range(6):
            warm_ps = psp.tile([P, 512], F32, name=f"warm_{w}", tag="ps")
            nc.tensor.matmul(warm_ps[:], scratch[:, 0:P], scratch[:],
                             start=True, stop=True)

        # ---- Loads ------------------------------------------------------
        # Three independent hw DMA queues (one per issuing engine):
        # sync + gpsimd stripe the x tensors (both queues in parallel),
        # scalar carries all the small weight/bias loads.
        w1sb = const.tile([P, KP1 * 2 * WIDTH], F8, name="w1sb", tag="w1sb")
        nc.scalar.dma_start(w1sb[:], w1_d.ap())
        w1v = w1sb[:].rearrange("p (j i c) -> p j i c", j=KP1, i=2)

        ball = const.tile([P, 2 * KB2], F32, name="ball", tag="ball")
        nc.scalar.dma_start(ball[:], b_d.ap())
        b1_t = ball[:, 0:KB2]
        b2_t = ball[:, KB2:2 * KB2]

        w2sb = const.tile([P, 9 * KB2 * WIDTH], F8, name="w2sb", tag="w2sb")
        nc.scalar.dma_start(w2sb[:], w2_d.ap())
        w2v = w2sb[:].rearrange("p (t i c) -> p t i c", t=9, i=2)

        w3sb = const.tile([P, KB2 * C_OUT], F8, name="w3sb", tag="w3sb")
        nc.scalar.dma_start(w3sb[:], w3_d.ap())
        w3v = w3sb[:].rearrange("p (i c) -> p i c", i=2)

        id_t = const.tile([P, P], BF16, name="id_t", tag="id_t")
        nc.scalar.dma_start(id_t[:], id_d.ap())

        xq = xpool.tile([P, KB1 * NLOC * HW], F8, name="xq", tag="xq")
        xqv = xq[:].rearrange("p (k n) -> p k n", k=KB1)
        CH = 2 * NLOC * HW  # chunk: 2 channel blocks, contiguous both sides
        xq_dmas = []
        for j in range(KB1 // 2):
            eng = nc.sync if j % 2 == 0 else nc.gpsimd
            i = eng.dma_start(xq[:, j * CH:(j + 1) * CH],
                              xq_d.ap()[:, j * CH:(j + 1) * CH])
            n = len(xq_dmas)
            if n >= 2:
                add_dep_helper(i.ins, xq_dmas[n - 2], reason="xq load pacing")
            xq_dmas.append(i.ins)

        # residual x (bf16, conv3 bias folded in), behind xq on both queues
        xr = xpool.tile([P, KB1 * NLOC * HW], BF16, name="xr", tag="xr")
        xrv = xr[:].rearrange("p (k n) -> p k n", k=KB1)
        xr_dmas = []
        for j in range(KB1 // 2):
            eng = nc.sync if j % 2 == 0 else nc.gpsimd
            i = eng.dma_start(xr[:, j * CH:(j + 1) * CH],
                              xr_d.ap()[:, j * CH:(j + 1) * CH])
            n = len(xr_dmas)
            if n < 2:
                add_dep_helper(i.ins, xq_dmas[2 + n], reason="xr behind xq")
            else:
                add_dep_helper(i.ins, xr_dmas[n - 2], reason="xr load pacing")
            xr_dmas.append(i.ins)

        # conv1 output: zero-padded row-interleaved pair fields, fp8.
        # Per image pair a 32x16 field (j = 2*padrow + img), payload at
        # j in 2..29, cols 1..14. Layout [P, KB2 * NPAIRS * FLD].
        # Only the pad cells are zeroed (3 small memsets, not the full
        # field): top/bottom pad rows and the left/right pad columns.
        out1 = opool.tile([P, KB2 * NPAIRS * FLD], F8, name="out1",
                          tag="out1")
        kq = KB2 * NPAIRS  # 16 fields, stride FLD
        fv = out1[:].rearrange("p (f j c) -> p f j c", f=kq, j=32, c=16)
        nc.vector.memset(fv[:, :, 0:2, :], 0.0)      # top pad rows j=0,1
        nc.vector.memset(fv[:, :, 30:32, :], 0.0)    # bottom pad rows
        nc.vector.memset(fv[:, :, 2:30, 0:1], 0.0)   # left pad col
        nc.vector.memset(fv[:, :, 2:30, 15:16], 0.0)  # right pad col
        out1v = out1[:].rearrange("p (k q j c) -> p k q j c",
                                  k=KB2, q=NPAIRS, j=32, c=16)

        out2 = opool.tile([P, KB2 * NLOC * HW], F8, name="out2", tag="out2")
        out2v = out2[:].rearrange("p (k n) -> p k n", k=KB2)

        # ---- conv1 (1x1, 1024->256) + bias + relu -> padded out1 --------
        # Per np-half: 8 open groups (4 npairs x 2 m), contraction kp outer.
        for half in range(2):
            nps = [half * 4 + j for j in range(4)]
            grp = {}
            for np_ in nps:
                for m in range(KB2):
                    grp[(np_, m)] = psp.tile([P, NF], F32,
                                             name=f"ps1_{np_}_{m}", tag="ps")
            for kp in range(KP1):
                for m in range(KB2):
                    for np_ in nps:
                        nc.tensor.matmul(
                            grp[(np_, m)][:],
                            w1v[:, kp, :, m * P:(m + 1) * P],
                            xqv[:, 2 * kp:2 * kp + 2,
                                np_ * NF:(np_ + 1) * NF],
                            start=(kp == 0), stop=(kp == KP1 - 1),
                            perf_mode=DR,
                        )
            for np_ in nps:
                for m in range(KB2):
                    dst = out1v[:, m, np_, 2:30, 1:15]
                    src = (grp[(np_, m)][:]
                           .rearrange("p (j c) -> p j c", j=28))
                    if np_ % 2 == 1:
                        nc.vector.tensor_scalar(dst, src, b1_t[:, m:m + 1],
                                                0.0, Alu.add, Alu.max)
                    else:
                        nc.scalar.activation(dst, src, Relu,
                                             bias=b1_t[:, m:m + 1])

        # ---- conv2 (3x3, 256->256, pad 1) + bias + relu -> out2 ----------
        # Per np-half: 8 open groups, contraction tap outer (each tap is a
        # DoubleRow pair over the two input channel blocks). The moving
        # operand is the shifted window over the interleaved padded field:
        # rows 2*dy..2*dy+27, cols dx..dx+13.
        for half in range(2):
            nps = [half * 4 + j for j in range(4)]
            grp = {}
            for np_ in nps:
                for m in range(KB2):
                    grp[(np_, m)] = psp.tile([P, NF], F32,
                                             name=f"ps2_{np_}_{m}", tag="ps")
            for t in range(9):
                dy, dx = t // 3, t % 3
                for m in range(KB2):
                    for np_ in nps:
                        rhs = out1v[:, :, np_, 2 * dy:2 * dy + 28,
                                    dx:dx + 14]
                        nc.tensor.matmul(
                            grp[(np_, m)][:]
                            .rearrange("p (j c) -> p j c", j=28),
                            w2v[:, t, :, m * P:(m + 1) * P],
                            rhs,
                            start=(t == 0), stop=(t == 8),
                            perf_mode=DR,
                        )
            for np_ in nps:
                for m in range(KB2):
                    dst = out2v[:, m, np_ * NF:(np_ + 1) * NF]
                    src = grp[(np_, m)][:]
                    if np_ % 2 == 1:
                        nc.vector.tensor_scalar(dst, src, b2_t[:, m:m + 1],
                                                0.0, Alu.add, Alu.max)
                    else:
                        nc.scalar.activation(dst, src, Relu,
                                             bias=b2_t[:, m:m + 1])

        # ---- conv3 (1x1, 256->1024) + residual + relu -> y --------------
        # Per m: 8 open groups as 4 pair tiles. Each group is one DoubleRow
        # matmul (K=256) plus a bf16 identity matmul (weights s3*I) adding
        # s3 * (x + b3) into PSUM; eviction is relu(psum)/s3 per pair.
        inv_s3 = 1.0 / S3
        for m in range(MB3):
            grp = {}
            for np_ in range(NPAIRS):
                grp[np_] = psp.tile([P, NF], F32, name=f"ps3_{np_}", tag="ps")
            for np_ in range(NPAIRS):
                nc.tensor.matmul(
                    grp[np_][:],
                    w3v[:, :, m * P:(m + 1) * P],
                    out2v[:, :, np_ * NF:(np_ + 1) * NF],
                    start=True, stop=False,
                    perf_mode=DR,
                )
            for np_ in range(NPAIRS):
                nc.tensor.matmul(
                    grp[np_][:], id_t[:],
                    xrv[:, m, np_ * NF:(np_ + 1) * NF],
                    start=False, stop=True,
                )
            ystage = evp.tile([P, NLOC * HW], BF16, name="ystage",
                              tag="ystage", bufs=3)
            for np_ in range(NPAIRS):
                dst = ystage[:, np_ * NF:(np_ + 1) * NF]
                if (np_ + m) % 2 == 1:
                    nc.vector.tensor_scalar(dst, grp[np_][:], 0.0, inv_s3,
                                            Alu.max, Alu.mult)
                else:
                    nc.scalar.activation(dst, grp[np_][:], Relu,
                                         bias=0.0, scale=inv_s3)
            nchunk = 4 if m == MB3 - 1 else 2
            CNF = NLOC * HW // nchunk
            for c in range(nchunk):
                eng = nc.sync if (m * 2 + c) % 2 == 0 else nc.gpsimd
                eng.dma_start(y_d.ap()[m][:, c * CNF:(c + 1) * CNF],
                              ystage[:, c * CNF:(c + 1) * CNF])


def _prep(x, w1, g1, b1, m1, v1, w2, g2, b2, m2, v2, w3, g3, b3, m3, v3):
    """Host-side: fold BN, scale + quantize weights to fp8, arrange SBUF
    images, shard + interleave x."""
    def fold(w, g, b, m, v):
        scale = (g.astype(np.float64) / np.sqrt(v.astype(np.float64) + EPS))
        bias = b.astype(np.float64) - m.astype(np.float64) * scale
        wf = w.astype(np.float64) * scale.reshape(-1, *([1] * (w.ndim - 1)))
        return wf.astype(np.float32), bias.astype(np.float32)

    w1f, bias1 = fold(w1, g1, b1, m1, v1)   # [256,1024,1,1]
    w2f, bias2 = fold(w2, g2, b2, m2, v2)   # [256,256,3,3]
    w3f, bias3 = fold(w3, g3, b3, m3, v3)   # [1024,256,1,1]

    f8 = ml_dtypes.float8_e4m3
    bf = ml_dtypes.bfloat16

    def q8(a):
        return np.clip(a, -240.0, 240.0).astype(f8)

    # w1 DoubleRow image [p, (kp i m)]: [p, kp, i, m] = w1f[m, (2kp+i)*128+p]
    w1t = np.ascontiguousarray(
        (w1f[:, :, 0, 0] * S1).T.reshape(KP1, 2, P, WIDTH)
        .transpose(2, 0, 1, 3).reshape(P, KP1 * 2 * WIDTH))
    # w2 image [p, (t i m)]: t = dy*3+dx, i = input block
    w2t = np.ascontiguousarray(
        (w2f * (S2 / S1)).transpose(2, 3, 1, 0)
        .reshape(9, KB2, P, WIDTH).transpose(2, 0, 1, 3)
        .reshape(P, 9 * KB2 * WIDTH))
    # w3 image [p, (i m)]
    w3t = np.ascontiguousarray(
        (w3f[:, :, 0, 0] * (S3 / S2)).T.reshape(KB2, P, C_OUT)
        .transpose(1, 0, 2).reshape(P, KB2 * C_OUT))

    b1h = (bias1 * S1).reshape(KB2, P).T                  # [P, 2]
    b2h = (bias2 * S2).reshape(KB2, P).T                  # [P, 2]
    ball = np.ascontiguousarray(
        np.concatenate([b1h, b2h], axis=1), dtype=np.float32)

    # x -> per-core partition-major [P, KB1*NLOC*HW], columns per k-block
    # ordered (pair, j=2r+i, c):
    # [core, pair, i, kb, p, r, c] -> [core, p, kb, pair, r, i, c]
    xs = (x.reshape(NCORES, NPAIRS, 2, KB1, P, 14, 14)
          .transpose(0, 4, 3, 1, 5, 2, 6)
          .reshape(NCORES, P, KB1 * NLOC * HW))
    xq = q8(xs)
    # residual: x + conv3 bias per channel, bf16
    xrf = x + bias3[None, :, None, None]
    xr = (xrf.reshape(NCORES, NPAIRS, 2, KB1, P, 14, 14)
          .transpose(0, 4, 3, 1, 5, 2, 6)
          .reshape(NCORES, P, KB1 * NLOC * HW)).astype(bf)

    ident = (np.eye(P, dtype=np.float32) * S3).astype(bf)

    common = {"w1t": q8(w1t), "w2t": q8(w2t), "w3t": q8(w3t),
              "biases": ball, "ident": ident}
    in_maps = [dict(common,
                    xq=np.ascontiguousarray(xq[i]),
                    xr=np.ascontiguousarray(xr[i]))
               for i in range(NCORES)]
    return in_maps


def kernel(**inputs):
    inputs = {k: np.asarray(v) for k, v in inputs.items()}
    in_maps = _prep(**inputs)
    nc = _build()
    res = run_bass_kernel_spmd(nc, in_maps, core_ids=list(range(NCORES)))

    y = np.empty((NCORES * NLOC, C_OUT, 14, 14), dtype=np.float32)
    for i in range(NCORES):
        r = np.asarray(res.results[i]["y"], dtype=np.float32)  # [MB3,P,N*HW]
        # columns are (pair, j=2r+i, c): [m, p, pair, r, i, c]
        r = (r.reshape(MB3, P, NPAIRS, 14, 2, 14)
             .transpose(2, 4, 0, 1, 3, 5)
             .reshape(NLOC, C_OUT, 14, 14))
        y[i * NLOC:(i + 1) * NLOC] = r
    return y
